# revision 1
# baseline (speedup 1.0000x reference)
"""Trainium2 Bass kernel for nn_Attention: single-head attention,
B=32, N=1024, DIM=512, fp32.

    q = X @ Wq.T ; k = X @ Wk.T ; v = X @ Wv.T
    out = softmax((q k^T)/sqrt(D)) @ v

Strategy (8 NeuronCores, data-parallel over batch, 4 batches/core):
  - Host folds A = (Wq.T @ Wk)/sqrt(D)  so scores = X A X.T  — saves one
    projection-sized matmul per batch and needs only X (transposed) on
    device.
  - All tensors live transposed on device: XT [d, n], GT = (X A).T,
    V [n, e], scores ST [k, q] (k on partitions).  Softmax runs along
    the partition axis: exp on ScalarE, partition sums via a ones-vector
    matmul, broadcast of 1/denom via a rank-1 ones matmul, normalization
    fused into the PSUM->SBUF eviction on VectorE.  Attention output is
    produced transposed (OT [e, q]) and the host transposes it back.
  - Matmuls use the float32r PE mode (full-rate fp32 streaming).
"""
import numpy as np

B, N, D = 32, 1024, 512
NCORES = 8
BPC = B // NCORES          # batches per core
DC = D // 128              # 4 chunks of 128 along d / e
KC = N // 128              # 8 chunks of 128 along k
QH = N // 512              # 2 q-halves of 512

_cache = {}


def _split_sync_waits(nc):
    """walrus on this image accepts at most ONE semaphore wait per
    instruction; hoist extras onto InstNoOp carriers on the same engine
    (same-engine program order preserves the gating)."""
    import concourse.mybir as mybir

    ctr = 0
    for f in nc.m.functions:
        for bb in f.blocks:
            out = []
            changed = False
            for ins in bb.instructions:
                si = getattr(ins, "sync_info", None)
                waits = list(si.on_wait) if si and si.on_wait else []
                if len(waits) > 1:
                    for w in waits[:-1]:
                        ctr += 1
                        out.append(
                            mybir.InstNoOp(
                                name=f"wsplit-{ctr}",
                                engine=ins.engine,
                                bass_nofuse=True,
                                sync_info=mybir.SyncInfo(on_wait=[w], on_update=[]),
                            )
                        )
                    ins.sync_info = mybir.SyncInfo(
                        on_wait=waits[-1:], on_update=list(si.on_update or [])
                    )
                    changed = True
                out.append(ins)
            if changed:
                bb.instructions[:] = out


def _build():
    import concourse.bass as bass
    import concourse.mybir as mybir
    import concourse.tile as tile

    f32 = mybir.dt.float32
    f32r = mybir.dt.float32r
    Exp = mybir.ActivationFunctionType.Exp

    nc = bass.Bass(target_bir_lowering=False)

    xt = nc.dram_tensor("xt", [BPC, D, N], f32, kind="ExternalInput")
    a_mat = nc.dram_tensor("a_mat", [D, D], f32, kind="ExternalInput")
    wvt = nc.dram_tensor("wvt", [D, D], f32, kind="ExternalInput")
    ones_col_d = nc.dram_tensor("ones_col", [128, 1], f32, kind="ExternalInput")
    ones_row_d = nc.dram_tensor("ones_row", [1, 128], f32, kind="ExternalInput")
    out_t = nc.dram_tensor("out_t", [BPC, D, N], f32, kind="ExternalOutput")

    with tile.TileContext(nc) as tc:
        with (
            tc.tile_pool(name="wpool", bufs=1) as wpool,
            tc.tile_pool(name="xpool", bufs=2) as xpool,
            tc.tile_pool(name="gpool", bufs=2) as gpool,
            tc.tile_pool(name="vpool", bufs=2) as vpool,
            tc.tile_pool(name="epool", bufs=3) as epool,
            tc.tile_pool(name="eapool", bufs=2) as eapool,
            tc.tile_pool(name="opool", bufs=2) as opool,
            tc.tile_pool(name="rpool", bufs=2) as rpool,
            tc.tile_pool(name="dpool", bufs=2) as dpool,
            tc.tile_pool(name="ps_ws", bufs=3, space="PSUM") as ps_ws,
            tc.tile_pool(name="ps_ot", bufs=4, space="PSUM") as ps_ot,
            tc.tile_pool(name="ps_den", bufs=1, space="PSUM") as ps_den,
        ):
            # --- weights / constants (once) ---
            a_sb = []
            for c in range(DC):
                t = wpool.tile([128, D], f32, tag=f"a{c}", name=f"a_sb{c}")
                nc.sync.dma_start(
                    t[:].bitcast(f32r),
                    a_mat[c * 128:(c + 1) * 128, :].bitcast(f32r),
                )
                a_sb.append(t)
            ones_col = wpool.tile([128, 1], f32, tag="onec")
            nc.sync.dma_start(ones_col[:].bitcast(f32r), ones_col_d[:].bitcast(f32r))
            ones_row = wpool.tile([1, 128], f32, tag="oner")
            nc.sync.dma_start(ones_row[:].bitcast(f32r), ones_row_d[:].bitcast(f32r))
            wvt_sb = []
            for c in range(DC):
                t = wpool.tile([128, D], f32, tag=f"wvt{c}", name=f"wvt_sb{c}")
                nc.sync.dma_start(
                    t[:].bitcast(f32r),
                    wvt[c * 128:(c + 1) * 128, :].bitcast(f32r),
                )
                wvt_sb.append(t)

            def load_xt(b):
                xts = []
                for c in range(DC):
                    t = xpool.tile([128, N], f32, tag=f"xt{c}", name=f"xt_b{b}c{c}")
                    nc.scalar.dma_start(
                        t[:].bitcast(f32r),
                        xt[b, c * 128:(c + 1) * 128, :].bitcast(f32r),
                    )
                    xts.append(t)
                return xts

            def gt_phase(b, xts):
                gt_sb = gpool.tile([128, DC * N], f32, tag="gt", name=f"gt_b{b}")
                for m in range(DC):
                    for h in range(QH):
                        pg = ps_ws.tile([128, 512], f32, tag="ws", name=f"pg{b}{m}{h}")
                        for k4 in range(DC):
                            nc.tensor.matmul(
                                pg[:],
                                a_sb[k4][:, m * 128:(m + 1) * 128].bitcast(f32r),
                                xts[k4][:, h * 512:(h + 1) * 512].bitcast(f32r),
                                start=(k4 == 0), stop=(k4 == DC - 1),
                            )
                        nc.scalar.copy(
                            gt_sb[:, m * N + h * 512:m * N + (h + 1) * 512].bitcast(f32r),
                            pg[:],
                        )
                return gt_sb

            def v_phase(b, xts):
                v_sb = vpool.tile([128, KC * D], f32, tag="v", name=f"v_b{b}")
                for m in range(KC):
                    pv = ps_ws.tile([128, 512], f32, tag="ws", name=f"pv{b}{m}")
                    for k4 in range(DC):
                        nc.tensor.matmul(
                            pv[:],
                            xts[k4][:, m * 128:(m + 1) * 128].bitcast(f32r),
                            wvt_sb[k4][:].bitcast(f32r),
                            start=(k4 == 0), stop=(k4 == DC - 1),
                        )
                    nc.scalar.copy(
                        v_sb[:, m * D:(m + 1) * D].bitcast(f32r), pv[:]
                    )
                return v_sb

            def gt_phase_k4outer(b, xts):
                gt_sb = gpool.tile([128, DC * N], f32, tag="gt", name=f"gt_b{b}")
                grp = {}
                for i, (m, h) in enumerate([(m, h) for m in range(DC) for h in range(QH)]):
                    pool = [ps_ws, ps_ws, ps_ws, ps_ot, ps_ot, ps_ot, ps_ot, ps_den][i]
                    grp[(m, h)] = pool.tile([128, 512], f32, tag=["ws", "ws", "ws", "ot", "ot", "ot", "ot", "den"][i], name=f"pg0_{m}{h}")
                for k4 in range(DC):
                    for m in range(DC):
                        for h in range(QH):
                            nc.tensor.matmul(
                                grp[(m, h)][:],
                                a_sb[k4][:, m * 128:(m + 1) * 128].bitcast(f32r),
                                xts[k4][:, h * 512:(h + 1) * 512].bitcast(f32r),
                                start=(k4 == 0), stop=(k4 == DC - 1),
                            )
                for m in range(DC):
                    for h in range(QH):
                        nc.scalar.copy(
                            gt_sb[:, m * N + h * 512:m * N + (h + 1) * 512].bitcast(f32r),
                            grp[(m, h)][:],
                        )
                return gt_sb

            def v_phase_k4outer(b, xts):
                v_sb = vpool.tile([128, KC * D], f32, tag="v", name=f"v_b{b}")
                grp = {}
                for m in range(KC):
                    pool = [ps_ws, ps_ws, ps_ws, ps_ot, ps_ot, ps_ot, ps_ot, ps_den][m]
                    grp[m] = pool.tile([128, 512], f32, tag=["ws", "ws", "ws", "ot", "ot", "ot", "ot", "den"][m], name=f"pv0_{m}")
                for k4 in range(DC):
                    for m in range(KC):
                        nc.tensor.matmul(
                            grp[m][:],
                            xts[k4][:, m * 128:(m + 1) * 128].bitcast(f32r),
                            wvt_sb[k4][:].bitcast(f32r),
                            start=(k4 == 0), stop=(k4 == DC - 1),
                        )
                for m in range(KC):
                    nc.scalar.copy(
                        v_sb[:, m * D:(m + 1) * D].bitcast(f32r), grp[m][:]
                    )
                return v_sb

            def ph2_compute(b, h, xts, gt_sb, v_sb):
                p_den = ps_den.tile([1, 512], f32, tag="den", name=f"den{b}{h}")
                p_ot = [ps_ot.tile([128, 512], f32, tag="ot", name=f"p_ot{b}{h}{m}")
                        for m in range(DC)]
                # E running sum on VectorE (replaces 7 of 8 ones-matmuls on PE)
                ea = [eapool.tile([128, 512], f32, tag="ea0", name=f"ea0_{b}{h}"),
                      eapool.tile([128, 512], f32, tag="ea1", name=f"ea1_{b}{h}")]
                for kc in range(KC):
                    p_st = ps_ws.tile([128, 512], f32, tag="ws", name=f"st{b}{h}{kc}")
                    for k4 in range(DC):
                        nc.tensor.matmul(
                            p_st[:],
                            xts[k4][:, kc * 128:(kc + 1) * 128].bitcast(f32r),
                            gt_sb[:, k4 * N + h * 512:k4 * N + (h + 1) * 512].bitcast(f32r),
                            start=(k4 == 0), stop=(k4 == DC - 1),
                        )
                    e_sb = epool.tile([128, 512], f32, tag="e", name=f"e{b}{h}{kc}")
                    nc.scalar.activation(e_sb[:].bitcast(f32r), p_st[:], Exp)
                    if kc == 0:
                        nc.vector.tensor_copy(ea[0][:], e_sb[:])
                    else:
                        nc.vector.tensor_add(
                            ea[kc % 2][:], ea[(kc + 1) % 2][:], e_sb[:]
                        )
                    for m in range(DC):
                        nc.tensor.matmul(
                            p_ot[m][:],
                            v_sb[:, kc * D + m * 128:kc * D + (m + 1) * 128].bitcast(f32r),
                            e_sb[:].bitcast(f32r),
                            start=(kc == 0), stop=(kc == KC - 1),
                        )
                ea_r = eapool.tile([128, 512], f32, tag="ear", name=f"ear{b}{h}")
                nc.vector.tensor_copy(ea_r[:].bitcast(f32r), ea[(KC - 1) % 2][:])
                nc.tensor.matmul(
                    p_den[:], ones_col[:].bitcast(f32r), ea_r[:].bitcast(f32r),
                    start=True, stop=True,
                )
                return p_den, p_ot

            def ph2_evict(b, h, p_den, p_ot):
                den_sb = dpool.tile([1, 512], f32, tag="densb", name=f"dsb{b}{h}")
                nc.vector.tensor_copy(den_sb[:].bitcast(f32r), p_den[:])
                otraw = opool.tile([128, DC * 512], f32, tag="otraw", name=f"orw{b}{h}")
                for m in range(DC):
                    nc.scalar.copy(otraw[:, m * 512:(m + 1) * 512], p_ot[m][:])
                return den_sb, otraw

            def ph2_norm(b, h, den_sb, otraw):
                p_bc = ps_ws.tile([128, 512], f32, tag="ws", name=f"bc{b}{h}")
                nc.tensor.matmul(
                    p_bc[:], ones_row[:].bitcast(f32r), den_sb[:].bitcast(f32r)
                )
                ln_sb = rpool.tile([128, 512], f32, tag="ln", name=f"ln{b}{h}")
                nc.scalar.activation(ln_sb[:], p_bc[:], mybir.ActivationFunctionType.Ln)
                rc_sb = rpool.tile([128, 512], f32, tag="rc", name=f"rc{b}{h}")
                nc.scalar.activation(rc_sb[:], ln_sb[:], mybir.ActivationFunctionType.Exp,
                                     scale=-1.0)
                ot_sb = opool.tile([128, DC * 512], f32, tag="ot", name=f"osb{b}{h}")
                for g in range(2):
                    for m in (2 * g, 2 * g + 1):
                        nc.vector.tensor_mul(
                            ot_sb[:, m * 512:(m + 1) * 512],
                            otraw[:, m * 512:(m + 1) * 512], rc_sb[:]
                        )
                    (nc.scalar if g == 0 else nc.sync).dma_start(
                        out_t[b, g * 256:(g + 1) * 256, h * 512:(h + 1) * 512].rearrange(
                            "(m p) q -> p m q", p=128
                        ),
                        ot_sb[:, g * 1024:(g + 1) * 1024].rearrange(
                            "p (m q) -> p m q", m=2
                        ),
                    )

            # software pipeline: phase-1 of batch b+1 fills the PE boundary
            # stalls of batch b's phase-2 (PE executes in program order).
            xts = load_xt(0)
            gt_sb = gt_phase_k4outer(0, xts)
            v_sb = v_phase_k4outer(0, xts)
            state = (xts, gt_sb, v_sb)
            for b in range(BPC):
                xts, gt_sb, v_sb = state
                p_den, p_ot = ph2_compute(b, 0, xts, gt_sb, v_sb)
                den_sb, otraw = ph2_evict(b, 0, p_den, p_ot)
                if b + 1 < BPC:
                    nxts = load_xt(b + 1)
                    ngt = gt_phase(b + 1, nxts)
                ph2_norm(b, 0, den_sb, otraw)
                p_den, p_ot = ph2_compute(b, 1, xts, gt_sb, v_sb)
                den_sb, otraw = ph2_evict(b, 1, p_den, p_ot)
                if b + 1 < BPC:
                    nv = v_phase(b + 1, nxts)
                    state = (nxts, ngt, nv)
                ph2_norm(b, 1, den_sb, otraw)
    return nc


def _prepare_inputs(embeddings, Wq, Wk, Wv):
    xt_all = np.ascontiguousarray(embeddings.transpose(0, 2, 1)).astype(
        np.float32, copy=False
    )
    a_mat = (
        Wq.astype(np.float64).T @ Wk.astype(np.float64) / np.sqrt(float(D))
    ).astype(np.float32)
    wvt = np.ascontiguousarray(Wv.T).astype(np.float32, copy=False)
    ones_col = np.ones((128, 1), np.float32)
    ones_row = np.ones((1, 128), np.float32)
    in_maps = []
    for i in range(NCORES):
        in_maps.append(
            {
                "xt": np.ascontiguousarray(xt_all[i * BPC:(i + 1) * BPC]),
                "a_mat": a_mat,
                "wvt": wvt,
                "ones_col": ones_col,
                "ones_row": ones_row,
            }
        )
    return in_maps


def _get_nc():
    if "nc" not in _cache:
        nc = _build()
        _split_sync_waits(nc)
        _cache["nc"] = nc
    return _cache["nc"]


def _assemble(results):
    out = np.empty((B, N, D), np.float32)
    for i in range(NCORES):
        ot = results[i]["out_t"]  # [BPC, D, N]
        out[i * BPC:(i + 1) * BPC] = ot.transpose(0, 2, 1)
    return out


def kernel(embeddings, Wq, Wk, Wv):
    from concourse.bass_utils import run_bass_kernel_spmd

    embeddings = np.asarray(embeddings, dtype=np.float32)
    in_maps = _prepare_inputs(
        embeddings, np.asarray(Wq), np.asarray(Wk), np.asarray(Wv)
    )
    res = run_bass_kernel_spmd(_get_nc(), in_maps, list(range(NCORES)))
    return _assemble(res.results)



# revision 6
# speedup vs baseline: 1.0171x; 1.0171x over previous
"""Trainium2 Bass kernel for nn_Attention: single-head attention,
B=32, N=1024, DIM=512, fp32 in/out.

    q = X @ Wq.T ; k = X @ Wk.T ; v = X @ Wv.T
    out = softmax((q k^T)/sqrt(D)) @ v

Strategy (8 NeuronCores, data-parallel over batch, 4 batches/core):
  - Host folds A = (Wq.T @ Wk)/sqrt(D)  so scores = X A X.T  — saves one
    projection-sized matmul per batch.
  - All matmul operands are bf16 (PSUM accumulation is fp32).  fp32r
    stationary loads cost ~224ns each and gate the PE at ~272ns/matmul;
    bf16 stationaries get fast weight load so the PE streams at the
    ~216ns/matmul roofline.
  - Layouts: XT [d, n] (bf16, host-prepped), GT = (X A).T [d->e, q],
    V [n(k), e], scores ST [k, q] (k on partitions).  Softmax runs
    along the partition axis: exp on ScalarE (bf16 out), running
    partial sums on VectorE (bf16, 2x rate), denominator broadcast to
    all partitions via an all-ones [128,128] stationary matmul
    accumulated in two chunks (partials + last exp tile) so the PE
    never waits on the vector chain.  1/den via ln+exp(-x) (same
    activation table).  Attention output is produced transposed
    (OT [e, q]); host transposes back.
  - Phase-2 is software-pipelined: S(kc+1) issues between S(kc) and
    O(kc) so exp latency is hidden.  PSUM: 2 S banks + 5 O banks +
    1 den bank = 8.
  - PSUM evictions run on GpSimd (otherwise idle); ScalarE keeps only
    exp/ln; DMA descriptors issue from SP.
  - ~48 tiny warmup matmuls at t=0 keep the PE HAM clock-gate warm
    through the initial DMA window (else first ~20us run at 1.2GHz).
"""
import numpy as np

B, N, D = 32, 1024, 512
NCORES = 8
BPC = B // NCORES          # batches per core
DC = D // 128              # 4 chunks of 128 along d / e
KC = N // 128              # 8 chunks of 128 along k
QH = N // 512              # 2 q-halves of 512

_cache = {}


def _split_sync_waits(nc):
    """walrus on this image accepts at most ONE semaphore wait per
    instruction; hoist extras onto InstNoOp carriers on the same engine
    (same-engine program order preserves the gating)."""
    import concourse.mybir as mybir

    ctr = 0
    for f in nc.m.functions:
        for bb in f.blocks:
            out = []
            changed = False
            for ins in bb.instructions:
                si = getattr(ins, "sync_info", None)
                waits = list(si.on_wait) if si and si.on_wait else []
                if len(waits) > 1:
                    for w in waits[:-1]:
                        ctr += 1
                        out.append(
                            mybir.InstNoOp(
                                name=f"wsplit-{ctr}",
                                engine=ins.engine,
                                bass_nofuse=True,
                                sync_info=mybir.SyncInfo(on_wait=[w], on_update=[]),
                            )
                        )
                    ins.sync_info = mybir.SyncInfo(
                        on_wait=waits[-1:], on_update=list(si.on_update or [])
                    )
                    changed = True
                out.append(ins)
            if changed:
                bb.instructions[:] = out


def _build():
    import concourse.bass as bass
    import concourse.mybir as mybir
    import concourse.tile as tile

    f32 = mybir.dt.float32
    bf16 = mybir.dt.bfloat16
    Exp = mybir.ActivationFunctionType.Exp
    Ln = mybir.ActivationFunctionType.Ln

    nc = bass.Bass(target_bir_lowering=False)

    xtb = nc.dram_tensor("xtb", [BPC, D, N], bf16, kind="ExternalInput")
    a_mat = nc.dram_tensor("a_mat", [D, D], bf16, kind="ExternalInput")
    wvt = nc.dram_tensor("wvt", [D, D], bf16, kind="ExternalInput")
    ones_bc_d = nc.dram_tensor("ones_bc", [128, 128], bf16, kind="ExternalInput")
    out_t = nc.dram_tensor("out_t", [BPC, D, N], f32, kind="ExternalOutput")

    with tile.TileContext(nc) as tc:
        with (
            tc.tile_pool(name="wpool", bufs=1) as wpool,
            tc.tile_pool(name="xpool", bufs=2) as xpool,
            tc.tile_pool(name="gpool", bufs=2) as gpool,
            tc.tile_pool(name="vpool", bufs=2) as vpool,
            tc.tile_pool(name="epool", bufs=4) as epool,
            tc.tile_pool(name="eapool", bufs=2) as eapool,
            tc.tile_pool(name="opool", bufs=2) as opool,
            tc.tile_pool(name="rpool", bufs=2) as rpool,
            tc.tile_pool(name="ps_ws", bufs=2, space="PSUM") as ps_ws,
            tc.tile_pool(name="ps_ot", bufs=5, space="PSUM") as ps_ot,
            tc.tile_pool(name="ps_den", bufs=1, space="PSUM") as ps_den,
        ):
            # --- warmup scratch (keep PE busy so HAM un-throttles while
            # the first input DMAs land) ---
            scr_st = wpool.tile([128, 128], bf16, tag="scr_st")
            scr_mv = wpool.tile([128, 128], bf16, tag="scr_mv")
            nc.vector.memset(scr_st[:], 0.0)
            nc.vector.memset(scr_mv[:], 0.0)

            # --- weights / constants (once) ---
            a_sb = []
            for c in range(DC):
                t = wpool.tile([128, D], bf16, tag=f"a{c}", name=f"a_sb{c}")
                nc.sync.dma_start(t[:], a_mat[c * 128:(c + 1) * 128, :])
                a_sb.append(t)
            ones_bc = wpool.tile([128, 128], bf16, tag="onebc")
            nc.sync.dma_start(ones_bc[:], ones_bc_d[:])
            wvt_sb = []
            for c in range(DC):
                t = wpool.tile([128, D], bf16, tag=f"wvt{c}", name=f"wvt_sb{c}")
                nc.sync.dma_start(t[:], wvt[c * 128:(c + 1) * 128, :])
                wvt_sb.append(t)

            # warmup matmuls: small moving dim (128) so they retire fast
            for w in range(48):
                pw = ps_ws.tile([128, 512], f32, tag="ws", name=f"wm{w}")
                nc.tensor.matmul(
                    pw[:, 0:128], scr_st[:], scr_mv[:], start=True, stop=True
                )

            def load_xtb(b):
                xts = []
                for c in range(DC):
                    t = xpool.tile([128, N], bf16, tag=f"xt{c}", name=f"xt_b{b}c{c}")
                    nc.sync.dma_start(t[:], xtb[b, c * 128:(c + 1) * 128, :])
                    xts.append(t)
                return xts

            def gt_phase(b, xts):
                gt_sb = gpool.tile([128, DC * N], bf16, tag="gt", name=f"gt_b{b}")
                for m in range(DC):
                    for h in range(QH):
                        pg = ps_ws.tile([128, 512], f32, tag="ws", name=f"pg{b}{m}{h}")
                        for k4 in range(DC):
                            nc.tensor.matmul(
                                pg[:],
                                a_sb[k4][:, m * 128:(m + 1) * 128],
                                xts[k4][:, h * 512:(h + 1) * 512],
                                start=(k4 == 0), stop=(k4 == DC - 1),
                            )
                        nc.scalar.copy(
                            gt_sb[:, m * N + h * 512:m * N + (h + 1) * 512], pg[:]
                        )
                return gt_sb

            def v_phase(b, xts):
                v_sb = vpool.tile([128, KC * D], bf16, tag="v", name=f"v_b{b}")
                for m in range(KC):
                    pv = ps_ws.tile([128, 512], f32, tag="ws", name=f"pv{b}{m}")
                    for k4 in range(DC):
                        nc.tensor.matmul(
                            pv[:],
                            xts[k4][:, m * 128:(m + 1) * 128],
                            wvt_sb[k4][:],
                            start=(k4 == 0), stop=(k4 == DC - 1),
                        )
                    nc.vector.tensor_copy(v_sb[:, m * D:(m + 1) * D], pv[:])
                return v_sb

            def ph2(b, h, xts, gt_sb, v_sb):
                """Full phase-2 block for one (batch, q-half): scores, softmax,
                attention output, normalization, output DMA."""
                p_den = ps_den.tile([128, 512], f32, tag="den", name=f"den{b}{h}")
                p_ot = [ps_ot.tile([128, 512], f32, tag="ot", name=f"p_ot{b}{h}{m}")
                        for m in range(DC)]
                ea = [eapool.tile([128, 512], bf16, tag=f"ea{i}", name=f"ea{i}_{b}{h}")
                      for i in range(2)]
                es = []

                def s_group(kc):
                    p_st = ps_ws.tile([128, 512], f32, tag="ws", name=f"st{b}{h}{kc}")
                    for k4 in range(DC):
                        nc.tensor.matmul(
                            p_st[:],
                            xts[k4][:, kc * 128:(kc + 1) * 128],
                            gt_sb[:, k4 * N + h * 512:k4 * N + (h + 1) * 512],
                            start=(k4 == 0), stop=(k4 == DC - 1),
                        )
                    e_sb = epool.tile([128, 512], bf16, tag="e", name=f"e{b}{h}{kc}")
                    nc.scalar.activation(e_sb[:], p_st[:], Exp)
                    # running partial sum of chunks 0..6 on VectorE (bf16)
                    if kc == 0:
                        nc.vector.tensor_copy(ea[0][:], e_sb[:])
                    elif kc < KC - 1:
                        nc.vector.tensor_add(ea[kc % 2][:], ea[(kc + 1) % 2][:], e_sb[:])
                    es.append(e_sb)

                def o_group(kc):
                    for m in range(DC):
                        nc.tensor.matmul(
                            p_ot[m][:],
                            v_sb[:, kc * D + m * 128:kc * D + (m + 1) * 128],
                            es[kc][:],
                            start=(kc == 0), stop=(kc == KC - 1),
                        )

                s_group(0)
                s_group(1)
                for kc in range(2, KC):
                    o_group(kc - 2)
                    s_group(kc)
                o_group(KC - 2)
                # den = sum_k E[k,q], broadcast to all 128 partitions via
                # all-ones stationary; two accumulation chunks so the second
                # has the same dependency as o_group(7) (no PE stall).
                nc.tensor.matmul(p_den[:], ones_bc[:], ea[(KC - 2) % 2][:],
                                 start=True, stop=False)
                o_group(KC - 1)
                nc.tensor.matmul(p_den[:], ones_bc[:], es[KC - 1][:],
                                 start=False, stop=True)

                # normalization chain (off PE): 1/den = exp(-ln(den))
                ln_sb = rpool.tile([128, 512], f32, tag="ln", name=f"ln{b}{h}")
                nc.scalar.activation(ln_sb[:], p_den[:], Ln)
                rc_sb = rpool.tile([128, 512], f32, tag="rc", name=f"rc{b}{h}")
                nc.scalar.activation(rc_sb[:], ln_sb[:], Exp, scale=-1.0)
                # normalize directly out of the p_ot PSUM banks (no
                # intermediate eviction); banks release one-by-one as the
                # muls retire, ps_ot bufs=5 gives the next block headroom.
                ot_sb = opool.tile([128, DC * 512], f32, tag="ot", name=f"osb{b}{h}")
                for g in range(2):
                    for m in (2 * g, 2 * g + 1):
                        nc.vector.tensor_mul(
                            ot_sb[:, m * 512:(m + 1) * 512],
                            p_ot[m][:], rc_sb[:]
                        )
                    nc.sync.dma_start(
                        out_t[b, g * 256:(g + 1) * 256, h * 512:(h + 1) * 512].rearrange(
                            "(m p) q -> p m q", p=128
                        ),
                        ot_sb[:, g * 1024:(g + 1) * 1024].rearrange(
                            "p (m q) -> p m q", m=2
                        ),
                    )

            # software pipeline across batches: phase-1 of batch b+1 fills
            # the PE boundary gaps of batch b's phase-2.
            xts = load_xtb(0)
            gt_sb = gt_phase(0, xts)
            v_sb = v_phase(0, xts)
            state = (xts, gt_sb, v_sb)
            for b in range(BPC):
                xts, gt_sb, v_sb = state
                if b + 1 < BPC:
                    nxts = load_xtb(b + 1)
                ph2(b, 0, xts, gt_sb, v_sb)
                if b + 1 < BPC:
                    ngt = gt_phase(b + 1, nxts)
                ph2(b, 1, xts, gt_sb, v_sb)
                if b + 1 < BPC:
                    nv = v_phase(b + 1, nxts)
                    state = (nxts, ngt, nv)
    return nc


def _prepare_inputs(embeddings, Wq, Wk, Wv):
    import ml_dtypes

    bf16 = ml_dtypes.bfloat16
    xt_all = np.ascontiguousarray(
        embeddings.transpose(0, 2, 1)
    ).astype(bf16)
    a_mat = (
        Wq.astype(np.float64).T @ Wk.astype(np.float64) / np.sqrt(float(D))
    ).astype(bf16)
    wvt = np.ascontiguousarray(Wv.T).astype(bf16)
    ones_bc = np.ones((128, 128), bf16)
    in_maps = []
    for i in range(NCORES):
        in_maps.append(
            {
                "xtb": np.ascontiguousarray(xt_all[i * BPC:(i + 1) * BPC]),
                "a_mat": a_mat,
                "wvt": wvt,
                "ones_bc": ones_bc,
            }
        )
    return in_maps


def _get_nc():
    if "nc" not in _cache:
        nc = _build()
        _split_sync_waits(nc)
        _cache["nc"] = nc
    return _cache["nc"]


def _assemble(results):
    out = np.empty((B, N, D), np.float32)
    for i in range(NCORES):
        ot = results[i]["out_t"]  # [BPC, D, N]
        out[i * BPC:(i + 1) * BPC] = ot.transpose(0, 2, 1)
    return out


def kernel(embeddings, Wq, Wk, Wv):
    from concourse.bass_utils import run_bass_kernel_spmd

    embeddings = np.asarray(embeddings, dtype=np.float32)
    in_maps = _prepare_inputs(
        embeddings, np.asarray(Wq), np.asarray(Wk), np.asarray(Wv)
    )
    res = run_bass_kernel_spmd(_get_nc(), in_maps, list(range(NCORES)))
    return _assemble(res.results)


# revision 9
# speedup vs baseline: 1.2103x; 1.1900x over previous
"""Trainium2 Bass kernel for nn_Attention: single-head attention,
B=32, N=1024, DIM=512, fp32 in/out.

    q = X @ Wq.T ; k = X @ Wk.T ; v = X @ Wv.T
    out = softmax((q k^T)/sqrt(D)) @ v

Strategy (8 NeuronCores, data-parallel over batch, 4 batches/core):
  - Host folds A = (Wq.T @ Wk)/sqrt(D) so scores = X A X.T — saves one
    projection-sized matmul per batch.
  - All matmul operands are bf16 (PSUM accumulation fp32): fp32r
    stationary loads cost ~224ns and gate the PE at ~272ns/matmul;
    bf16 loads are ~116ns and hide under the 213ns moving stream.
  - PSUM tiles are [128,1024] pairs (2 banks) so each ScalarE exp /
    eviction covers two 512-wide matmul groups: halves ScalarE op
    count.  PSUM: 2x ws-pair + 2x ot-pair = 8 banks; the softmax
    denominator tile shares the ws rotation.
  - Softmax along partitions: exp on ScalarE (bf16 out), partial sums
    on VectorE, denominator broadcast to all partitions via an
    all-ones [128,128] stationary matmul accumulated in three chunks
    (sum of exp chunks 0-5, then exp chunks 6 and 7 directly) so the
    PE never waits on the vector chain.  1/den = exp(-ln(den)) (both
    in one activation table).  Normalization multiplies read the
    attention accumulators straight out of PSUM (no eviction).
  - Phase-1 (G = (XA)^T and V) of batch b+1 is sliced into 8 pair-
    groups used as fillers between phase-2 stages of batch b, so
    single-rotation PSUM WAR gaps are always covered; the last batch
    uses dummy scratch matmuls as fillers.
  - Tile puts a tick-semaphore increment on every PE instruction
    (~26ns each); _coalesce_tick_updates defers increments within
    wait-free runs onto accumulation-group stops.
  - Warmup matmuls at t=0 keep the PE HAM clock-gate warming while
    the first input DMAs land (else the first ~15us run at 1.2GHz).
"""
import numpy as np

B, N, D = 32, 1024, 512
NCORES = 8
BPC = B // NCORES          # batches per core
DC = D // 128              # 4 chunks of 128 along d / e
KC = N // 128              # 8 chunks of 128 along k
QH = N // 512              # 2 q-halves of 512

_cache = {}


def _split_sync_waits(nc):
    """walrus on this image accepts at most ONE semaphore wait per
    instruction; hoist extras onto InstNoOp carriers on the same engine
    (same-engine program order preserves the gating)."""
    import concourse.mybir as mybir

    ctr = 0
    for f in nc.m.functions:
        for bb in f.blocks:
            out = []
            changed = False
            for ins in bb.instructions:
                si = getattr(ins, "sync_info", None)
                waits = list(si.on_wait) if si and si.on_wait else []
                if len(waits) > 1:
                    for w in waits[:-1]:
                        ctr += 1
                        out.append(
                            mybir.InstNoOp(
                                name=f"wsplit-{ctr}",
                                engine=ins.engine,
                                bass_nofuse=True,
                                sync_info=mybir.SyncInfo(on_wait=[w], on_update=[]),
                            )
                        )
                    ins.sync_info = mybir.SyncInfo(
                        on_wait=waits[-1:], on_update=list(si.on_update or [])
                    )
                    changed = True
                out.append(ins)
            if changed:
                bb.instructions[:] = out


def _coalesce_tick_updates(nc):
    """Every PE matmul carries a +1 update on the engine tick semaphore
    (~26ns of EVT_SEM write each).  Within a run of consecutive PE
    matmuls that carry no waits, defer the increments onto the run's
    boundary instructions (accumulation-group stops / the instruction
    before the next wait), summing the values.  Waits elsewhere observe
    the same cumulative counts no later than the carrier's retirement,
    and carriers are wait-free so they always retire: no deadlock."""
    import concourse.mybir as mybir

    for f in nc.m.functions:
        for bb in f.blocks:
            pend = 0
            carrier = None       # last stripped instruction
            carrier_upd = None   # its (stripped) SyncUpdate

            def flush():
                nonlocal pend, carrier, carrier_upd
                if carrier is not None and pend > 0:
                    carrier_upd.update_value = pend
                    carrier.sync_info = mybir.SyncInfo(
                        on_wait=list(carrier.sync_info.on_wait or []),
                        on_update=[carrier_upd],
                    )
                pend = 0
                carrier = None
                carrier_upd = None

            for ins in bb.instructions:
                if ins.engine != mybir.EngineType.PE:
                    continue
                si = getattr(ins, "sync_info", None)
                waits = list(si.on_wait) if si and si.on_wait else []
                ups = list(si.on_update) if si and si.on_update else []
                tick_only = (
                    type(ins).__name__ == "InstMatmult"
                    and len(ups) == 1
                    and ups[0].sync_type == "semaphore"
                    and ups[0].update_mode == "sem-inc"
                    and ups[0].update_reg is None
                )
                if waits:
                    flush()
                if not tick_only:
                    flush()
                    continue
                # strip the update, remember it
                ins.sync_info = mybir.SyncInfo(on_wait=waits, on_update=[])
                pend += ups[0].update_value
                carrier = ins
                carrier_upd = ups[0]
                if getattr(ins, "stop_tensor_calc", False):
                    flush()
            flush()


def _build():
    import concourse.bass as bass
    import concourse.mybir as mybir
    import concourse.tile as tile

    f32 = mybir.dt.float32
    bf16 = mybir.dt.bfloat16
    Exp = mybir.ActivationFunctionType.Exp
    Ln = mybir.ActivationFunctionType.Ln

    nc = bass.Bass(target_bir_lowering=False)

    xtb = nc.dram_tensor("xtb", [BPC, D, N], bf16, kind="ExternalInput")
    a_mat = nc.dram_tensor("a_mat", [D, D], bf16, kind="ExternalInput")
    wvt = nc.dram_tensor("wvt", [D, D], bf16, kind="ExternalInput")
    ones_bc_d = nc.dram_tensor("ones_bc", [128, 128], bf16, kind="ExternalInput")
    out_t = nc.dram_tensor("out_t", [BPC, D, N], f32, kind="ExternalOutput")

    with tile.TileContext(nc) as tc:
        with (
            tc.tile_pool(name="wpool", bufs=1) as wpool,
            tc.tile_pool(name="xpool", bufs=3) as xpool,
            tc.tile_pool(name="gpool", bufs=2) as gpool,
            tc.tile_pool(name="vpool", bufs=2) as vpool,
            tc.tile_pool(name="epool", bufs=3) as epool,
            tc.tile_pool(name="eapool", bufs=2) as eapool,
            tc.tile_pool(name="opool", bufs=2) as opool,
            tc.tile_pool(name="rpool", bufs=2) as rpool,
            tc.tile_pool(name="ps_ws", bufs=2, space="PSUM") as ps_ws,
            tc.tile_pool(name="ps_ot", bufs=2, space="PSUM") as ps_ot,
        ):
            # --- scratch for warmup + dummy fillers ---
            scr_st = wpool.tile([128, 128], bf16, tag="scr_st")
            scr_mv = wpool.tile([128, 512], bf16, tag="scr_mv")
            nc.vector.memset(scr_st[:], 0.0)
            nc.vector.memset(scr_mv[:], 0.0)

            # --- weights / constants; order DMAs so batch-0 needs land first ---
            a_sb = []
            for c in range(DC):
                t = wpool.tile([128, D], bf16, tag=f"a{c}", name=f"a_sb{c}")
                nc.sync.dma_start(t[:], a_mat[c * 128:(c + 1) * 128, :])
                a_sb.append(t)

            def load_xtb(b):
                xts = []
                for c in range(DC):
                    t = xpool.tile([128, N], bf16, tag=f"xt{c}", name=f"xt_b{b}c{c}")
                    nc.sync.dma_start(t[:], xtb[b, c * 128:(c + 1) * 128, :])
                    xts.append(t)
                return xts

            xts0 = load_xtb(0)
            ones_bc = wpool.tile([128, 128], bf16, tag="onebc")
            nc.sync.dma_start(ones_bc[:], ones_bc_d[:])
            wvt_sb = []
            for c in range(DC):
                t = wpool.tile([128, D], bf16, tag=f"wvt{c}", name=f"wvt_sb{c}")
                nc.sync.dma_start(t[:], wvt[c * 128:(c + 1) * 128, :])
                wvt_sb.append(t)
            xts1 = load_xtb(1)

            # --- warmup: PE busy from t=0 so HAM un-throttles before real work
            pw = ps_ws.tile([128, 1024], f32, tag="ws", name="warm_ps")
            for w in range(28):
                nc.tensor.matmul(
                    pw[:, 0:256], scr_st[:], scr_mv[:, 0:256],
                    start=True, stop=True,
                )

            def g_group(b, m, xts, gt_sb):
                """G^T pair-group: both q-halves for one e-chunk m."""
                pg = ps_ws.tile([128, 1024], f32, tag="ws", name=f"pg{b}{m}")
                for k4 in range(DC):
                    for h in range(QH):
                        nc.tensor.matmul(
                            pg[:, h * 512:(h + 1) * 512],
                            a_sb[k4][:, m * 128:(m + 1) * 128],
                            xts[k4][:, h * 512:(h + 1) * 512],
                            start=(k4 == 0), stop=(k4 == DC - 1),
                            skip_group_check=True,
                        )
                nc.scalar.copy(gt_sb[:, m * N:m * N + 1024], pg[:])

            def v_group(b, j, xts, v_sb):
                """V pair-group: key-chunks 2j and 2j+1."""
                pv = ps_ws.tile([128, 1024], f32, tag="ws", name=f"pv{b}{j}")
                for k4 in range(DC):
                    for half in range(2):
                        kc = 2 * j + half
                        nc.tensor.matmul(
                            pv[:, half * 512:(half + 1) * 512],
                            xts[k4][:, kc * 128:(kc + 1) * 128],
                            wvt_sb[k4][:],
                            start=(k4 == 0), stop=(k4 == DC - 1),
                            skip_group_check=True,
                        )
                nc.scalar.copy(v_sb[:, 2 * j * 512:(2 * j + 2) * 512], pv[:])

            def ph2(b, h, xts, gt_sb, v_sb, fillers):
                """Phase-2 for one (batch, q-half): scores, softmax,
                attention output, normalization, output DMA.  `fillers`
                is a list of 4 callables (phase-1 pair-groups of the
                next batch) slotted between stages to cover PSUM WAR
                rotation gaps; missing ones become dummy matmuls."""
                p_ot = [ps_ot.tile([128, 1024], f32, tag="ot",
                                   name=f"p_ot{b}{h}{i}") for i in range(2)]
                es = []
                p_den = [None]

                def dummy8(target):
                    for _ in range(8):
                        nc.tensor.matmul(target, scr_st[:], scr_mv[:],
                                         start=True, stop=True,
                                         skip_group_check=True)

                def fill(i):
                    if i < len(fillers):
                        fillers[i]()
                    elif i == 0:
                        dummy8(p_ot[0][:, 0:512])
                    else:
                        dummy8(p_den[0][:, 512:1024])

                def s_pair(p):
                    p_st = ps_ws.tile([128, 1024], f32, tag="ws",
                                      name=f"st{b}{h}{p}")
                    for half in range(2):
                        kc = 2 * p + half
                        for k4 in range(DC):
                            nc.tensor.matmul(
                                p_st[:, half * 512:(half + 1) * 512],
                                xts[k4][:, kc * 128:(kc + 1) * 128],
                                gt_sb[:, k4 * N + h * 512:k4 * N + (h + 1) * 512],
                                start=(k4 == 0), stop=(k4 == DC - 1),
                                skip_group_check=True,
                            )
                    e_sb = epool.tile([128, 1024], bf16, tag="e",
                                      name=f"e{b}{h}{p}")
                    nc.scalar.activation(e_sb[:], p_st[:], Exp)
                    es.append(e_sb)

                def o_pair(p):
                    for half in range(2):
                        kc = 2 * p + half
                        for m in range(DC):
                            nc.tensor.matmul(
                                p_ot[m // 2][:, (m % 2) * 512:(m % 2 + 1) * 512],
                                v_sb[:, kc * 512 + m * 128:kc * 512 + (m + 1) * 128],
                                es[p][:, half * 512:(half + 1) * 512],
                                start=(kc == 0), stop=(kc == KC - 1),
                                skip_group_check=True,
                            )

                s_pair(0)
                fill(0)
                s_pair(1)
                o_pair(0)
                # running softmax-denominator partials on VectorE (bf16)
                ea01 = eapool.tile([128, 1024], bf16, tag="ea01", name=f"ea01_{b}{h}")
                nc.vector.tensor_add(ea01[:], es[0][:], es[1][:])
                s_pair(2)
                o_pair(1)
                ea012 = eapool.tile([128, 1024], bf16, tag="ea012", name=f"ea012_{b}{h}")
                nc.vector.tensor_add(ea012[:], ea01[:], es[2][:])
                easum = eapool.tile([128, 512], bf16, tag="easum", name=f"eas{b}{h}")
                nc.vector.tensor_add(easum[:], ea012[:, 0:512], ea012[:, 512:1024])
                s_pair(3)
                o_pair(2)
                # den = sum_k E[k,q] broadcast to all partitions via all-ones
                # stationary, accumulated in 3 chunks; the last two share
                # o_pair(3)'s exp dependency so the PE never stalls here.
                pd = ps_ws.tile([128, 1024], f32, tag="ws", name=f"den{b}{h}")
                p_den[0] = pd
                nc.tensor.matmul(pd[:, 0:512], ones_bc[:], easum[:],
                                 start=True, stop=False, skip_group_check=True)
                fill(1)
                o_pair(3)
                fill(2)
                nc.tensor.matmul(pd[:, 0:512], ones_bc[:], es[3][:, 0:512],
                                 start=False, stop=False, skip_group_check=True)
                nc.tensor.matmul(pd[:, 0:512], ones_bc[:], es[3][:, 512:1024],
                                 start=False, stop=True, skip_group_check=True)
                # normalization chain (off PE): 1/den = exp(-ln(den))
                ln_sb = rpool.tile([128, 512], f32, tag="ln", name=f"ln{b}{h}")
                nc.scalar.activation(ln_sb[:], pd[:, 0:512], Ln)
                rc_sb = rpool.tile([128, 512], f32, tag="rc", name=f"rc{b}{h}")
                nc.scalar.activation(rc_sb[:], ln_sb[:], Exp, scale=-1.0)
                ot_sb = opool.tile([128, DC * 512], f32, tag="ot", name=f"osb{b}{h}")
                for g in range(2):
                    for m in (2 * g, 2 * g + 1):
                        nc.vector.tensor_mul(
                            ot_sb[:, m * 512:(m + 1) * 512],
                            p_ot[m // 2][:, (m % 2) * 512:(m % 2 + 1) * 512],
                            rc_sb[:],
                        )
                    nc.sync.dma_start(
                        out_t[b, g * 256:(g + 1) * 256, h * 512:(h + 1) * 512].rearrange(
                            "(m p) q -> p m q", p=128
                        ),
                        ot_sb[:, g * 1024:(g + 1) * 1024].rearrange(
                            "p (m q) -> p m q", m=2
                        ),
                    )
                fill(3)

            # --- batch 0 phase-1 standalone (ws rotation covers the gaps) ---
            gt0 = gpool.tile([128, DC * N], bf16, tag="gt", name="gt_b0")
            for m in range(DC):
                g_group(0, m, xts0, gt0)
            v0 = vpool.tile([128, KC * 512], bf16, tag="v", name="v_b0")
            for j in range(KC // 2):
                v_group(0, j, xts0, v0)

            xts = {0: xts0, 1: xts1}
            gt = {0: gt0}
            v = {0: v0}
            for b in range(BPC):
                if b + 2 < BPC:
                    xts[b + 2] = load_xtb(b + 2)
                fillers_g = []
                fillers_v = []
                if b + 1 < BPC:
                    gt[b + 1] = gpool.tile([128, DC * N], bf16, tag="gt",
                                           name=f"gt_b{b+1}")
                    v[b + 1] = vpool.tile([128, KC * 512], bf16, tag="v",
                                          name=f"v_b{b+1}")
                    fillers_g = [
                        (lambda m=m: g_group(b + 1, m, xts[b + 1], gt[b + 1]))
                        for m in range(DC)
                    ]
                    fillers_v = [
                        (lambda j=j: v_group(b + 1, j, xts[b + 1], v[b + 1]))
                        for j in range(KC // 2)
                    ]
                ph2(b, 0, xts[b], gt[b], v[b], fillers_g)
                ph2(b, 1, xts[b], gt[b], v[b], fillers_v)
    return nc


def _prepare_inputs(embeddings, Wq, Wk, Wv):
    import ml_dtypes

    bf16 = ml_dtypes.bfloat16
    xt_all = np.ascontiguousarray(
        embeddings.transpose(0, 2, 1)
    ).astype(bf16)
    a_mat = (
        Wq.astype(np.float64).T @ Wk.astype(np.float64) / np.sqrt(float(D))
    ).astype(bf16)
    wvt = np.ascontiguousarray(Wv.T).astype(bf16)
    ones_bc = np.ones((128, 128), bf16)
    in_maps = []
    for i in range(NCORES):
        in_maps.append(
            {
                "xtb": np.ascontiguousarray(xt_all[i * BPC:(i + 1) * BPC]),
                "a_mat": a_mat,
                "wvt": wvt,
                "ones_bc": ones_bc,
            }
        )
    return in_maps


def _get_nc():
    if "nc" not in _cache:
        import os

        nc = _build()
        if not os.environ.get("BASS_NO_COALESCE"):
            _coalesce_tick_updates(nc)
        _split_sync_waits(nc)
        _cache["nc"] = nc
    return _cache["nc"]


def _assemble(results):
    out = np.empty((B, N, D), np.float32)
    for i in range(NCORES):
        ot = results[i]["out_t"]  # [BPC, D, N]
        out[i * BPC:(i + 1) * BPC] = ot.transpose(0, 2, 1)
    return out


def kernel(embeddings, Wq, Wk, Wv):
    from concourse.bass_utils import run_bass_kernel_spmd

    embeddings = np.asarray(embeddings, dtype=np.float32)
    in_maps = _prepare_inputs(
        embeddings, np.asarray(Wq), np.asarray(Wk), np.asarray(Wv)
    )
    res = run_bass_kernel_spmd(_get_nc(), in_maps, list(range(NCORES)))
    return _assemble(res.results)
